# revision 35
# baseline (speedup 1.0000x reference)
"""DSAttention (de-stationary attention) Trainium2 Bass kernel, v4.

Sharding: 8 cores; core c handles batch b=c//2, heads h0=4*(c%2) .. h0+4.
Each core computes its batch's tau/delta projectors redundantly (2 cores per
batch), then 4 independent causal-attention heads.

Host-side prep (layout/dtype only): q/k transposed to (e, l) fp16 head-pair
tiles, v_seq pre-interleaved fp16, V pre-arranged (s%128, h, s//128, e+1) fp16
with a ones column for the rowsum trick, weights pre-chunked fp16 and the two
projectors' conv taps packed together.

Math per (b,h):
  scores^T[s,l] = sum_e K[s,e] Q[l,e]          (s on partitions)
  A = softmax(tau/8 * scores + delta/8)        (causal)
    = exp(c1*qk - SHIFT) * g[s] / rowsum,  c1 = tau/8, g = exp(delta/8)
The g factor and denominator fold into the AV matmul by scaling V rows by g
and the appended ones-column (psum row 64 = rowsum).  Output is written as
(e, l) fp16 after on-device divide; host only transposes back.

Scheduling: the exp stream on the Activation engine is the bottleneck
(~0.83 ns/column, ~70 us total), so everything is arranged to start it early
and keep it dense: stats use ACT squares + PE matmul accumulation instead of
DVE fold chains, activation-table loads are prefetched by dummy ops, the tau MLP is emitted just before the first QK span and
the delta MLP (only needed by AV) after the second span.  QK matmuls for the
two heads of a pair sit at partition bases 0/64 and are emitted interleaved,
so they run concurrently in disjoint PE row-group quadrants.  Causal masks
and AV blocks are emitted as soon as their stream spans are emitted, so the
PE's static order interleaves AV work into its stream slack.
"""

import math
import sys
from contextlib import ExitStack

import numpy as np

sys.path.insert(0, "/opt/trn_rl_repo")

import concourse.bass as bass
import concourse.bacc as bacc
import concourse.tile as tile
from concourse import mybir
from concourse.bass_utils import run_bass_kernel_spmd

F32 = mybir.dt.float32
F16 = mybir.dt.float16
AF = mybir.ActivationFunctionType

B, L, H, E = 4, 2048, 8, 64
S = L
HE = H * E          # 512
DM = 512
NCORES = 8
HEADS_PER_CORE = 4
NT = S // 128                    # 16 s-tiles
SHIFT = 5.0                      # constant exp shift (cancels in softmax)
LOG8 = math.log(8.0)
SPAN = 1536                      # psum span (3 banks) per head of a pair

# stream offsets: A^T row j (len 2048-128j) packed back to back
ROW_LEN = [L - 128 * j for j in range(NT)]
ROW_OFF = [0] * NT
for _j in range(1, NT):
    ROW_OFF[_j] = ROW_OFF[_j - 1] + ROW_LEN[_j - 1]
STREAM_LEN = ROW_OFF[-1] + ROW_LEN[-1]          # 17408


def _stream_spans():
    """Spans of the packed causal stream.  Each span is (base, width, segs);
    segs = (off_in_span, j, l0, ln) cut at 512 psum-bank boundaries."""
    spans = []
    pos = 0
    cur = []
    cur_base = 0
    for j in range(NT):
        l = 128 * j
        while l < L:
            off = pos - cur_base
            take = min(SPAN - off, L - l)
            o, ll, rem = off, l, take
            while rem > 0:
                t2 = min(rem, 512 - o % 512)
                cur.append((o, j, ll, t2))
                o += t2
                ll += t2
                rem -= t2
            pos += take
            l += take
            if pos - cur_base == SPAN:
                spans.append((cur_base, SPAN, cur))
                cur = []
                cur_base = pos
    if cur:
        spans.append((cur_base, pos - cur_base, cur))
    return spans


def build_program():
    nc = bacc.Bacc("TRN2", target_bir_lowering=False, debug=False,
                   num_devices=NCORES)

    qt_d = nc.dram_tensor("qt", (2, 128, L), F16, kind="ExternalInput")
    kt_d = nc.dram_tensor("kt", (2, 128, S), F16, kind="ExternalInput")
    vseq_d = nc.dram_tensor("vseq", (128, NT, 512), F16, kind="ExternalInput")
    vat_d = nc.dram_tensor("vat", (128, HEADS_PER_CORE, NT, 65), F16,
                           kind="ExternalInput")
    conv_d = {p: nc.dram_tensor(p + "_conv", (128, NT, 3), F16,
                                kind="ExternalInput") for p in ("tau", "delta")}
    wd = {}
    for p in ("tau", "delta"):
        wd[p + "_w1"] = nc.dram_tensor(p + "_w1", (128, 8, DM), F16,
                                       kind="ExternalInput")
        wd[p + "_w2"] = nc.dram_tensor(p + "_w2", (128, 4, DM // 2), F16,
                                       kind="ExternalInput")
        wd[p + "_w3"] = nc.dram_tensor(p + "_w3", (128, 2, DM // 4), F16,
                                       kind="ExternalInput")
    wd["tau_w4"] = nc.dram_tensor("tau_w4", (DM // 4, 1), F16, kind="ExternalInput")
    wd["delta_w4"] = nc.dram_tensor("delta_w4", (DM // 4, S), F16,
                                    kind="ExternalInput")
    out_d = nc.dram_tensor("outT", (HEADS_PER_CORE, 64, L), F16,
                           kind="ExternalOutput")

    with tile.TileContext(nc) as tc, ExitStack() as octx:
        const = octx.enter_context(tc.tile_pool(name="const", bufs=1))
        ident1 = const.tile([1, 1], F32)
        nc.vector.memset(ident1, 1.0)
        invscol = const.tile([128, 1], F16)
        nc.vector.memset(invscol, 1.0 / S)
        # causal mask for diagonal 128x128 blocks of A^T (s part, l free):
        # keep where l >= s, zero elsewhere
        cmask = const.tile([128, 128], F16)
        nc.vector.memset(cmask, 1.0)
        nc.gpsimd.affine_select(
            out=cmask[:, :], in_=cmask[:, :],
            compare_op=mybir.AluOpType.is_ge, fill=0.0,
            base=0, channel_multiplier=-1, pattern=[[1, 128]])
        c1_bc = const.tile([128, 1], F32)
        g_sb = const.tile([128, NT], F32)
        # register const APs used as float biases in activations
        for val in (0.0, 1e-5, -LOG8, -SHIFT):
            ct = const.tile([128, 1], F32, tag=f"const{val}")
            nc.vector.memset(ct, val)
            nc.const_aps.aps[(F32, val)] = ct[:, :]

        # activation-table warmup: the first ACT op triggers a table-set
        # load (~1.3us); do a dummy Sqrt now so squares/copies/sqrt all hit
        # the resident sqrt set, off the critical path
        dum = const.tile([1, 1], F32)
        nc.scalar.activation(dum[:, :], ident1[:, :], AF.Sqrt, bias=1e-5)

        # tensors that outlive the projector scope
        inpool = octx.enter_context(tc.tile_pool(name="inputs", bufs=1))
        vat = inpool.tile([128, HEADS_PER_CORE, NT, 65], F16, tag="vat")
        qt_sb, kt_sb, w_sb = [], [], {}
        xT_p = {}

        # ---------------- projector conv + stats (scoped pools) ------------
        with ExitStack() as pctx:
            ppsum = pctx.enter_context(
                tc.tile_pool(name="proj_psum", bufs=2, space="PSUM"))
            psb = pctx.enter_context(tc.tile_pool(name="proj_sb", bufs=1))
            vpool = pctx.enter_context(tc.tile_pool(name="vseq", bufs=1))

            # PE clock warmup: ~2.5us of dummy matmuls while the DMAs run,
            # so the conv/stats chain starts at full PE clock (the PE ramps
            # to full speed only after ~3us of sustained activity)
            warm = ppsum.tile([128, 128], F32, tag="warm", bufs=1)
            for i in range(24):
                nc.tensor.matmul(warm[:, :], cmask[:, :], cmask[:, :],
                                 start=(i == 0), stop=(i == 23))

            vseq = vpool.tile([128, NT, 512], F16, tag="vseq")
            # conv taps spread to free cols 0/32/64 so the matmul puts
            # its three output rows on 32-aligned psum partitions (HW rule:
            # engine APs must start at 32-aligned partitions)
            cw_sb = {}
            for p in ("tau", "delta"):
                cwc = psb.tile([128, NT, 3], F16, tag=p + "cwc", name="cwc")
                nc.sync.dma_start(out=cwc[:, :, :], in_=conv_d[p].ap())
                cw = psb.tile([128, NT, 96], F16, tag=p + "cw", name="cw")
                nc.gpsimd.memset(cw[:, :, :], 0.0)
                nc.gpsimd.tensor_copy(out=cw[:, :, 0:96:32], in_=cwc[:, :, :])
                cw_sb[p] = cw

            def vseq_dma(q):
                nc.sync.dma_start(out=vseq[:, 4 * q : 4 * q + 4, :],
                                  in_=vseq_d.ap()[:, 4 * q : 4 * q + 4, :])
            vseq_dma(0)
            for p in ("tau", "delta"):
                w1 = inpool.tile([128, 8, DM], F16, tag=p + "w1")
                w2 = inpool.tile([128, 4, DM // 2], F16, tag=p + "w2")
                w3 = inpool.tile([128, 2, DM // 4], F16, tag=p + "w3")
                w4 = inpool.tile([128, 1 if p == "tau" else S], F16, tag=p + "w4")
                w_sb[p] = (w1, w2, w3, w4)
            for hp in range(2):
                qt = inpool.tile([128, L], F16, tag=f"qt{hp}")
                qt_sb.append(qt)
                kt = inpool.tile([128, S], F16, tag=f"kt{hp}")
                kt_sb.append(kt)
            # DMA order = need order, with vseq quarters interleaved so the
            # conv/stats pipeline and the tau weights progress together:
            # tau weights gate c1, pair-0 q/k gate the first QK spans
            vseq_dma(1)
            vseq_dma(2)
            vseq_dma(3)
            w1, w2, w3, w4 = w_sb["tau"]
            nc.sync.dma_start(out=w1[:, :, :], in_=wd["tau_w1"].ap())
            nc.sync.dma_start(out=w2[:, :, :], in_=wd["tau_w2"].ap())
            nc.sync.dma_start(out=w3[:, :, :], in_=wd["tau_w3"].ap())
            nc.sync.dma_start(out=w4[:, :], in_=wd["tau_w4"].ap())
            nc.sync.dma_start(out=kt_sb[0][:, :], in_=kt_d.ap()[0])
            nc.sync.dma_start(out=qt_sb[0][:, :], in_=qt_d.ap()[0])
            w1, w2, w3, w4 = w_sb["delta"]
            nc.sync.dma_start(out=w1[:, :, :], in_=wd["delta_w1"].ap())
            nc.sync.dma_start(out=w2[:, :, :], in_=wd["delta_w2"].ap())
            nc.sync.dma_start(out=w3[:, :, :], in_=wd["delta_w3"].ap())
            nc.sync.dma_start(out=w4[:, :], in_=wd["delta_w4"].ap())
            nc.sync.dma_start(out=kt_sb[1][:, :], in_=kt_d.ap()[1])
            nc.sync.dma_start(out=qt_sb[1][:, :], in_=qt_d.ap()[1])
            nc.sync.dma_start(out=vat[:, :, :, :], in_=vat_d.ap())

            # conv for BOTH projectors in one accumulation: c3 rows 0-2 tau
            # taps, rows 3-5 delta taps.  Stats: ACT squares, one DVE fold
            # level, then (1/S)-stationary matmuls accumulate the rest.
            c3 = {p: ppsum.tile([96, 512], F32, tag=p + "c3", name="c3",
                              bufs=1)
                  for p in ("tau", "delta")}
            sqt = vpool.tile([128, NT, 512], F16, tag="sqt")
            vf1 = vpool.tile([128, NT, 256], F16, tag="vf1")
            sf1 = vpool.tile([128, NT, 256], F16, tag="sf1")
            mean_row = psb.tile([1, 512], F32, tag="meanrow")
            def extract_xc(pi, p, eng):
                # circular shift-add of the three conv-tap psum rows.
                # xc[j] = r0[j-1] + r1[j] + r2[j+1].  Engines may read only
                # one PSUM operand per op, so the middle row is staged to
                # SBUF via the (otherwise idle) scalar engine first.
                cp = c3[p]
                r1s = psb.tile([1, 512], F32, tag=p + "r1", name="r1s")
                nc.scalar.copy(r1s[:, :], cp[32:33, :])
                xcp = psb.tile([1, 512], F32, tag=p + "xc", name="xcp")
                eng.tensor_add(xcp[:, 1:512], cp[0:1, 0:511], r1s[:, 1:512])
                eng.tensor_add(xcp[:, 0:1], cp[0:1, 511:512], r1s[:, 0:1])
                eng.tensor_add(xcp[:, 0:511], xcp[:, 0:511], cp[64:65, 1:512])
                eng.tensor_add(xcp[:, 511:512], xcp[:, 511:512], cp[64:65, 0:1])
                return xcp

            xc = {}
            # conv matmuls first and contiguous: the c3 chain ends right
            # after the last vseq quarter lands, so xc (on tau's critical
            # path to c1) is ready early; the stats pipeline follows
            for t in range(NT):
                nc.tensor.matmul(c3["tau"][:, :], cw_sb["tau"][:, t, :],
                                 vseq[:, t, :],
                                 start=(t == 0), stop=(t == NT - 1))
                nc.tensor.matmul(c3["delta"][:, :], cw_sb["delta"][:, t, :],
                                 vseq[:, t, :],
                                 start=(t == 0), stop=(t == NT - 1))
            mean_ps = ppsum.tile([1, 512], F32, tag="meanps", bufs=1)
            esq_ps = ppsum.tile([1, 512], F32, tag="esqps", bufs=1)
            for q in range(4):
                ts = slice(4 * q, 4 * q + 4)
                # squares split across ACT and DVE so the last quarter's
                # square isn't serialized behind three ACT squares
                if q < 2:
                    nc.scalar.activation(sqt[:, ts, :], vseq[:, ts, :],
                                         AF.Square)
                else:
                    nc.vector.tensor_mul(sqt[:, ts, :], vseq[:, ts, :],
                                         vseq[:, ts, :])
                nc.vector.tensor_add(vf1[:, ts, :], vseq[:, ts, 0:256],
                                     vseq[:, ts, 256:512])
                nc.vector.tensor_add(sf1[:, ts, :], sqt[:, ts, 0:256],
                                     sqt[:, ts, 256:512])
                for h in (2 * q, 2 * q + 1):
                    for (src, dst) in ((vf1, mean_ps), (sf1, esq_ps)):
                        # all heads accumulate into one (1,512) psum row at
                        # disjoint columns -> a single copy out per row
                        n = 0
                        for t in (2 * h, 2 * h + 1):
                            for so2 in range(4):
                                nc.tensor.matmul(
                                    dst[:, 64 * h : 64 * (h + 1)],
                                    invscol[:, :],
                                    src[:, t, 64 * so2 : 64 * so2 + 64],
                                    start=(n == 0), stop=(n == 7),
                                    skip_group_check=True)
                                n += 1
            # mean to SBUF (also feeds delta's xT); esq consumed from psum.
            # DVE queue order: mean copy -> mean^2 -> xc adds -> var sub, so
            # the mean^2 runs while the esq stats matmuls finish
            nc.vector.tensor_copy(out=mean_row[:, :], in_=mean_ps[:, :])
            var_row = psb.tile([1, 512], F32, tag="var")
            nc.vector.tensor_mul(var_row[:, :], mean_row[:, :], mean_row[:, :])
            xc["tau"] = extract_xc(0, "tau", nc.vector)

            xc["delta"] = extract_xc(1, "delta", nc.vector)

            nc.vector.tensor_sub(var_row[:, :], esq_ps[:, :], var_row[:, :])
            std_row = psb.tile([1, 512], F32, tag="stdrow")
            nc.scalar.activation(std_row[:, :], var_row[:, :], AF.Sqrt,
                                 bias=1e-5)
            # dummy Exp: pulls the exp table-set load here, where it hides
            # under the PE's xT/MLP work instead of stalling the first exp
            nc.scalar.activation(dum[:, :], ident1[:, :], AF.Exp)

            # x^T chunks (128, 8) per projector: cols 0-3 conv, 4-7 stats
            for p, stat in (("tau", std_row), ("delta", mean_row)):
                xT = inpool.tile([128, 8], F16, tag=p + "xT")
                for ci, row in ((0, xc[p]), (4, stat)):
                    tp = ppsum.tile([128, 4], F32, tag="pptp")
                    for m in range(4):
                        nc.tensor.transpose(tp[:, m : m + 1],
                                            row[:, m * 128 : (m + 1) * 128],
                                            ident1)
                    nc.vector.tensor_copy(out=xT[:, ci : ci + 4], in_=tp[:, :])
                xT_p[p] = xT

        # ---------------- attention phase ----------------------------------
        with ExitStack() as actx:
            qsA_pool = actx.enter_context(
                tc.tile_pool(name="qsA", bufs=1, space="PSUM"))
            qsB_pool = actx.enter_context(
                tc.tile_pool(name="qsB", bufs=1, space="PSUM"))
            av_psum = actx.enter_context(
                tc.tile_pool(name="av_psum", bufs=2, space="PSUM"))
            apool = actx.enter_context(tc.tile_pool(name="atiles", bufs=2))
            opool = actx.enter_context(tc.tile_pool(name="outsb", bufs=3))
            rcpool = actx.enter_context(tc.tile_pool(name="rc", bufs=2))
            bcpool = actx.enter_context(tc.tile_pool(name="bc", bufs=2))
            msb = actx.enter_context(tc.tile_pool(name="mlp_sb", bufs=1))

            def mlp(p, pt):
                """4-layer MLP of projector p; pt() -> fresh (128,512) psum."""
                xT = xT_p[p]
                w1, w2, w3, w4 = w_sb[p]
                r1 = pt()[0:1, 0:512]
                for m in range(8):
                    nc.tensor.matmul(r1, xT[:, m : m + 1], w1[:, m, :],
                                     start=(m == 0), stop=(m == 7))
                h1 = msb.tile([1, 512], F32, tag=p + "h1")
                if p == "tau":
                    nc.scalar.activation(h1[:, :], r1, AF.Relu)
                else:
                    nc.vector.tensor_scalar_max(h1[:, :], r1, 0.0)
                h1T = msb.tile([128, 4], F16, tag=p + "h1T")
                tp = pt()[:, 0:4]
                for m in range(4):
                    nc.tensor.transpose(tp[:, m : m + 1],
                                        h1[:, m * 128 : (m + 1) * 128], ident1)
                nc.vector.tensor_copy(out=h1T[:, :], in_=tp)

                r2 = pt()[0:1, 0:256]
                for m in range(4):
                    nc.tensor.matmul(r2, h1T[:, m : m + 1], w2[:, m, :],
                                     start=(m == 0), stop=(m == 3))
                h2 = msb.tile([1, 256], F32, tag=p + "h2")
                if p == "tau":
                    nc.scalar.activation(h2[:, :], r2, AF.Relu)
                else:
                    nc.vector.tensor_scalar_max(h2[:, :], r2, 0.0)
                h2T = msb.tile([128, 2], F16, tag=p + "h2T")
                tp = pt()[:, 0:2]
                for m in range(2):
                    nc.tensor.transpose(tp[:, m : m + 1],
                                        h2[:, m * 128 : (m + 1) * 128], ident1)
                nc.vector.tensor_copy(out=h2T[:, :], in_=tp)

                r3 = pt()[0:1, 0:128]
                for m in range(2):
                    nc.tensor.matmul(r3, h2T[:, m : m + 1], w3[:, m, :],
                                     start=(m == 0), stop=(m == 1))
                h3 = msb.tile([1, 128], F32, tag=p + "h3")
                if p == "tau":
                    nc.scalar.activation(h3[:, :], r3, AF.Relu)
                else:
                    nc.vector.tensor_scalar_max(h3[:, :], r3, 0.0)
                h3T = msb.tile([128, 1], F16, tag=p + "h3T")
                tp = pt()[:, 0:1]
                nc.tensor.transpose(tp[:, 0:1], h3[:, 0:128], ident1)
                nc.vector.tensor_copy(out=h3T[:, :], in_=tp)

                if p == "tau":
                    rt = pt()[0:1, 0:1]
                    nc.tensor.matmul(rt, h3T[:, :], w4[:, :],
                                     start=True, stop=True)
                    c1 = msb.tile([1, 1], F32, tag="c1s")
                    # c1 = exp(logit - ln 8) = tau / 8
                    nc.scalar.activation(c1[:, :], rt, AF.Exp, bias=-LOG8)
                    nc.gpsimd.partition_broadcast(c1_bc[:, :], c1[:, :])
                else:
                    dl = pt()[:, 0:16]
                    for m in range(NT):
                        nc.tensor.matmul(dl[:, m : m + 1],
                                         w4[:, m * 128 : (m + 1) * 128],
                                         h3T[:, :], start=True, stop=True)
                    # g = exp(delta / 8)
                    nc.scalar.activation(g_sb[:, :], dl, AF.Exp, scale=0.125)
                    # scale V (and its ones column) by g per s-tile
                    for t in range(NT):
                        nc.vector.tensor_scalar_mul(vat[:, :, t, :],
                                                    vat[:, :, t, :],
                                                    g_sb[:, t : t + 1])

            def av_tile():
                return av_psum.tile([128, 512], F32, tag="avp", name="avp")

            # tau MLP first: c1 gates the exp stream; borrows the AV psum
            # rotation (first AV block comes much later)
            mlp("tau", av_tile)

            spans = _stream_spans()
            atiles = {}
            covered = [0] * HEADS_PER_CORE     # stream pos exp'd per head
            next_mask = [0] * HEADS_PER_CORE   # next diag row to mask
            next_av = [0] * HEADS_PER_CORE     # next l-block to emit

            # AV l-blocks are emitted in two chunks: the j < 4*lb rows
            # (full-width reads, ready early) accumulate first; the last
            # four diag rows + the normalize/store epilogue follow once the
            # stream reaches them.  This removes most of the end-of-stream
            # AV tail.  avp accumulation groups interleave with span
            # matmuls, so group checks are skipped (different psum banks).
            av_state = {}   # h -> open avp tile

            def emit_av_chunk(h, lb, chunk):
                at = atiles[h]
                jmax = min(NT, 4 * lb + 4)
                if chunk == 0:
                    js = range(0, 4 * lb)
                else:
                    js = range(4 * lb, jmax)
                    if lb == 0:
                        js = range(0, jmax)
                if chunk == 0:
                    avp = av_tile()
                    av_state[h] = avp
                else:
                    avp = av_state.pop(h) if lb > 0 else av_tile()
                for j in js:
                    l0 = max(128 * j, 512 * lb)
                    l1 = 512 * (lb + 1)
                    sp0 = ROW_OFF[j] + (l0 - 128 * j)
                    nc.tensor.matmul(
                        avp[0:65, l0 - 512 * lb : l1 - 512 * lb],
                        vat[:, h, j, :],
                        at[:, sp0 : sp0 + (l1 - l0)],
                        start=(j == 0), stop=(chunk == 1 and j == jmax - 1),
                        skip_group_check=True)
                if chunk == 0:
                    return
                rc = rcpool.tile([1, 512], F32, tag="rc")
                nc.vector.reciprocal(out=rc[:, :], in_=avp[64:65, 0:512])
                bc = bcpool.tile([64, 512], F32, tag="bc")
                nc.gpsimd.partition_broadcast(bc[:, :], rc[:, :])
                osb = opool.tile([64, 512], F16, tag="osb")
                nc.vector.tensor_mul(osb[:, :], avp[0:64, 0:512], bc[:, :])
                nc.sync.dma_start(
                    out=out_d.ap()[h, :, 512 * lb : 512 * (lb + 1)],
                    in_=osb[:, :])

            def av_need(lb, chunk):
                if chunk == 0:      # j in [0, 4lb), widest requirement at last
                    return ROW_OFF[4 * lb - 1] + 512 * (lb + 1) - 128 * (4 * lb - 1)
                jlast = min(NT, 4 * lb + 4) - 1
                return ROW_OFF[jlast] + 512 * (lb + 1) - 128 * jlast

            drain_rr = [0]

            def drain_ready(max_av=1):
                # emit masks/AV chunks whose stream dependencies are emitted,
                # so the PE's static order has AV work between QK spans.
                # max_av rate-limits AV chunks per span slot so the PE backlog
                # never starves the ACT exp stream of fresh QK spans; the
                # start head rotates so no head's blocks pile up at the end.
                n_av = 0
                drain_rr[0] += 1
                for hi in range(HEADS_PER_CORE):
                    h = (hi + drain_rr[0]) % HEADS_PER_CORE
                    if h not in atiles:
                        continue
                    at = atiles[h]
                    while (next_mask[h] < NT
                           and covered[h] >= ROW_OFF[next_mask[h]] + 128):
                        j = next_mask[h]
                        nc.vector.tensor_mul(
                            at[:, ROW_OFF[j] : ROW_OFF[j] + 128],
                            at[:, ROW_OFF[j] : ROW_OFF[j] + 128], cmask[:, :])
                        next_mask[h] += 1
                    while next_av[h] < 8 and n_av < max_av:
                        lb, chunk = divmod(next_av[h], 2)
                        if lb == 0 and chunk == 0:
                            next_av[h] += 1
                            continue
                        jlast = min(NT, 4 * lb + 4) - 1
                        if chunk == 1 and next_mask[h] <= jlast:
                            break
                        if covered[h] >= av_need(lb, chunk):
                            emit_av_chunk(h, lb, chunk)
                            next_av[h] += 1
                            n_av += 1
                        else:
                            break

            nspan_emitted = 0
            for hp in range(2):
                qt, kt = qt_sb[hp], kt_sb[hp]
                atA = apool.tile([128, STREAM_LEN], F16, tag="atA")
                atB = apool.tile([128, STREAM_LEN], F16, tag="atB")
                atiles[2 * hp] = atA
                atiles[2 * hp + 1] = atB
                for (base, width, segs) in spans:
                    qsA = qsA_pool.tile([128, SPAN], F32, tag="qsA")
                    qsB = qsB_pool.tile([128, SPAN], F32, tag="qsB")
                    for (o, j, l0, ln) in segs:
                        # two heads at PE row-groups 0-1 / 2-3: concurrent
                        nc.tensor.matmul(qsA[:, o : o + ln],
                                         kt[0:64, 128 * j : 128 * (j + 1)],
                                         qt[0:64, l0 : l0 + ln],
                                         start=True, stop=True)
                        nc.tensor.matmul(qsB[:, o : o + ln],
                                         kt[64:128, 128 * j : 128 * (j + 1)],
                                         qt[64:128, l0 : l0 + ln],
                                         start=True, stop=True)
                    nc.scalar.activation(atA[:, base : base + width],
                                         qsA[:, 0:width], AF.Exp,
                                         bias=-SHIFT, scale=c1_bc[:, 0:1])
                    nc.scalar.activation(atB[:, base : base + width],
                                         qsB[:, 0:width], AF.Exp,
                                         bias=-SHIFT, scale=c1_bc[:, 0:1])
                    covered[2 * hp] = base + width
                    covered[2 * hp + 1] = base + width
                    nspan_emitted += 1
                    if nspan_emitted == 3:
                        # delta MLP: g only gates AV; borrow the AV psum tag
                        mlp("delta", av_tile)
                    drain_ready(max_av=2 if nspan_emitted > 20 else 1)
            drain_ready(max_av=64)
            assert all(n == 8 for n in next_av)

    nc.compile()
    return nc


def prepare_inmaps(inputs):
    """Host-side sharding + layout/dtype transforms (no math)."""
    q = np.asarray(inputs["queries"], dtype=np.float32)
    k = np.asarray(inputs["keys"], dtype=np.float32)
    v = np.asarray(inputs["values"], dtype=np.float32)

    wmap = {}
    for p in ("tau", "delta"):
        cw = np.asarray(inputs[p + "_conv_w"], np.float32)[0]        # (2048, 3)
        wmap[p + "_conv"] = np.ascontiguousarray(
            cw.reshape(NT, 128, 3).transpose(1, 0, 2).astype(np.float16))
        w1 = np.asarray(inputs[p + "_w1"], np.float32)               # (1024, 512)
        wmap[p + "_w1"] = np.ascontiguousarray(
            w1.reshape(8, 128, DM).transpose(1, 0, 2).astype(np.float16))
        w2 = np.asarray(inputs[p + "_w2"], np.float32)               # (512, 256)
        wmap[p + "_w2"] = np.ascontiguousarray(
            w2.reshape(4, 128, DM // 2).transpose(1, 0, 2).astype(np.float16))
        w3 = np.asarray(inputs[p + "_w3"], np.float32)               # (256, 128)
        wmap[p + "_w3"] = np.ascontiguousarray(
            w3.reshape(2, 128, DM // 4).transpose(1, 0, 2).astype(np.float16))
        wmap[p + "_w4"] = np.ascontiguousarray(
            np.asarray(inputs[p + "_w4"], np.float32).astype(np.float16))
    in_maps = []
    for c in range(NCORES):
        b = c // 2
        h0 = HEADS_PER_CORE * (c % 2)
        # qT/kT head-pair tiles: rows 0-63 head 2hp, 64-127 head 2hp+1
        qts = np.empty((2, 128, L), np.float16)
        kts = np.empty((2, 128, S), np.float16)
        for hp in range(2):
            for hl in range(2):
                h = h0 + 2 * hp + hl
                qts[hp, 64 * hl : 64 * hl + 64] = q[b, :, h, :].T
                kts[hp, 64 * hl : 64 * hl + 64] = k[b, :, h, :].T
        # v_seq interleaved: row i of v_seq = values[8*(i%256):+8, i//256, :]
        vs = v[b].reshape(256, 8, 8, 64).transpose(2, 0, 1, 3).reshape(2048, 512)
        vseq = vs.reshape(NT, 128, 512).transpose(1, 0, 2)
        # V for AV with ones column: vat[p, h, t, e] = v[b, 128t+p, h0+h, e]
        vat = np.ones((128, HEADS_PER_CORE, NT, 65), np.float16)
        vat[:, :, :, :64] = v[b, :, h0 : h0 + HEADS_PER_CORE, :].reshape(
            NT, 128, HEADS_PER_CORE, 64).transpose(1, 2, 0, 3)
        im = {
            "qt": qts,
            "kt": kts,
            "vseq": np.ascontiguousarray(vseq.astype(np.float16)),
            "vat": np.ascontiguousarray(vat),
        }
        im.update(wmap)
        in_maps.append(im)
    return in_maps


_CACHE = {}


def _get_program():
    if "nc" not in _CACHE:
        _CACHE["nc"] = build_program()
    return _CACHE["nc"]


def kernel(**inputs):
    nc = _get_program()
    in_maps = prepare_inmaps(inputs)
    res = run_bass_kernel_spmd(nc, in_maps, core_ids=list(range(NCORES)))
    full = np.empty((B, L, H, E), dtype=np.float32)
    for c in range(NCORES):
        b = c // 2
        h0 = HEADS_PER_CORE * (c % 2)
        outT = np.asarray(res.results[c]["outT"], np.float32)   # (4, 64, 2048)
        full[b, :, h0 : h0 + HEADS_PER_CORE, :] = outT.transpose(2, 0, 1)
    return full
